# revision 35
# baseline (speedup 1.0000x reference)
"""MoE (top-2 of 8 experts) Trainium2 Bass kernel, data-parallel over tokens.

16384 tokens sharded 2048/core across 8 NeuronCores. Per core:
  R. router logits via f32r matmuls (full fp32-class accuracy at bf16
     speed), top-2 via DVE max/max_index, gate weights sigmoid(l1-l2).
  P. compacted slot positions via PE prefix-sum matmuls (f16 one-hots);
     (token id, gate weight) PAIRS scattered into a slot-ordered DRAM
     list, one indirect DMA per (block, choice) with [P,1] offsets (the
     HW SWDGE ucode only honors single-column offset APs); padding
     slots pre-filled with the sentinel token id T (a dedicated zero
     row of the gather source / dump row of the output).
  E. per expert e: 5 per-chunk indirect gathers of its <=640 token rows
     (bf16), PE-transpose to [d, tok], W1 matmul -> gelu+b1 (scalar),
     W2 matmul -> gate-weight-scaled f32 y rows (vector drain), then 5
     per-chunk indirect scatter-ADD DMAs (cce add) accumulate the rows
     straight into the output tensor (serialized across experts - a
     token's two experts share its output row). No combine phase.
  B. the b2 term sum_e comb[t,e]*b2[e,:] is a tiny [T,E]@[E,D] matmul
     (comb = gate-weight one-hot mix) written to out as its base value
     before the scatter-adds begin.

DMA queues: SP HWDGE streams the 33.5MB of expert weights; Act HWDGE
carries router strips + fills + binit writes; the Pool SWDGE queue
carries scatters, readbacks, gathers and the serialized scatter-adds.
"""

import sys

if "/opt/trn_rl_repo" not in sys.path:
    sys.path.insert(0, "/opt/trn_rl_repo")

import ml_dtypes
import numpy as np

import concourse.bass as bass
import concourse.mybir as mybir
import concourse.tile as tile
from concourse.bass import IndirectOffsetOnAxis
from concourse.bass_utils import run_bass_kernel_spmd
from concourse.masks import make_identity, make_upper_triangular

f32 = mybir.dt.float32
f32r = mybir.dt.float32r
f16 = mybir.dt.float16
bf16 = mybir.dt.bfloat16
i32 = mybir.dt.int32
u32 = mybir.dt.uint32
Alu = mybir.AluOpType
Act = mybir.ActivationFunctionType

P = 128
N_CORES = 8
B, L, D, E = 4, 4096, 1024, 8
T = (B * L) // N_CORES      # tokens per core
NB = T // P                 # 128-token blocks per core
KD = D // P                 # contraction chunks
C = 640                     # per-(core, expert) token capacity
TC = C // P                 # gathered 128-token chunks per expert
SENT = T                    # padding token id -> zero row of xb / dump row of out


def _split_multi_waits(nc):
    """walrus here supports one semaphore wait per instruction; hoist
    extra waits onto single-wait NOPs just before the instruction."""
    ctr = 0
    for f in nc.m.functions:
        for bb in f.blocks:
            old = list(bb.instructions)
            new = []
            changed = False
            for inst in old:
                si = getattr(inst, "sync_info", None)
                waits = list(si.on_wait) if si is not None and si.on_wait else []
                if len(waits) > 1:
                    changed = True
                    for w in waits[:-1]:
                        ctr += 1
                        nop = mybir.InstNoOp(
                            name=f"I-waitsplit-{ctr}",
                            sync_info=mybir.SyncInfo(on_wait=[w], on_update=[]),
                            bass_nofuse=True,
                            engine=inst.engine,
                        )
                        nc.register_instruction(nop, overwrite=True)
                        new.append(nop)
                    del si.on_wait[:-1]
                new.append(inst)
            if changed:
                bb.instructions = new
    return ctr


def _build():
    nc = bass.Bass("TRN2", num_devices=N_CORES, num_swdge_queues=4)

    xT = nc.declare_dram_parameter("xT", [D, T], f32, isOutput=False)
    xb = nc.declare_dram_parameter("xb", [T + 1, D], bf16, isOutput=False)
    wrr = nc.declare_dram_parameter("wrr", [P, KD * E], f32, isOutput=False)
    w1t = nc.declare_dram_parameter("w1t", [E, D, D], bf16, isOutput=False)
    w2t = nc.declare_dram_parameter("w2t", [E, D, D], bf16, isOutput=False)
    b1d = nc.declare_dram_parameter("b1d", [P, E * KD], f32, isOutput=False)
    b2e = nc.declare_dram_parameter("b2e", [E, D], f16, isOutput=False)
    out = nc.declare_dram_parameter("out", [T + 1, D], bf16, isOutput=True)

    gall2 = nc.dram_tensor("gall2", [E * C, 2], f32)

    with tile.TileContext(nc) as tc:
        with (
            tc.tile_pool(name="persist", bufs=1) as pp,
            tc.tile_pool(name="w1p", bufs=2) as w1p,
            tc.tile_pool(name="w2p", bufs=2) as w2p,
        ):
            # ---- constants ----
            ident_f32 = pp.tile([P, P], f32, tag="idf32")
            make_identity(nc, ident_f32[:])
            ident_bf = pp.tile([P, P], bf16, tag="idbf")
            make_identity(nc, ident_bf[:])
            ident_f16 = pp.tile([P, P], f16, tag="idf16")
            make_identity(nc, ident_f16[:])
            u128 = pp.tile([P, P], f16, tag="u128")
            make_upper_triangular(nc, u128[:], val=1.0, diag=True)
            u16s = pp.tile([16, 16], f16, tag="u16s")
            make_upper_triangular(nc, u16s[:], val=1.0, diag=False)

            iota_e_i = pp.tile([P, NB * E], i32, tag="iotaei")
            nc.gpsimd.iota(
                iota_e_i[:], pattern=[[0, NB], [1, E]], base=0, channel_multiplier=0
            )
            iota_e = pp.tile([P, NB * E], f16, tag="iotae")
            nc.vector.tensor_copy(out=iota_e[:], in_=iota_e_i[:])
            tok2i = pp.tile([P, 2 * NB], i32, tag="tok2i")
            nc.gpsimd.iota(
                tok2i[:], pattern=[[0, 2], [P, NB]], base=0, channel_multiplier=1
            )
            pairs = pp.tile([P, 2 * NB * 2], f32, tag="pairs")
            nc.vector.tensor_copy(
                out=pairs[:].rearrange("p (c two) -> p c two", two=2)[:, :, 0],
                in_=tok2i[:],
            )
            c_fill = pp.tile([P, E * TC * 2], f32, tag="cfill")
            nc.vector.memset(
                c_fill[:].rearrange("p (c two) -> p c two", two=2)[:, :, 0], SENT
            )
            nc.vector.memset(
                c_fill[:].rearrange("p (c two) -> p c two", two=2)[:, :, 1], 0.0
            )

            b1_sb = pp.tile([P, E * KD], f32, tag="b1sb")
            nc.scalar.dma_start(out=b1_sb[:], in_=b1d[:])
            b2_sb = pp.tile([E, D], f16, tag="b2sb")
            nc.scalar.dma_start(out=b2_sb[:], in_=b2e[:])
            wr_sb = pp.tile([P, KD * E], f32, tag="wrsb")
            nc.scalar.dma_start(out=wr_sb[:], in_=wrr[:])

            # sentinel-fill the slot list (padding slots never written)
            gall_fill = nc.scalar.dma_start(
                out=gall2.rearrange("(c p) t -> p c t", p=P),
                in_=c_fill[:].rearrange("p (c two) -> p c two", two=2),
            )

            # ---- persistent routing state ----
            gl_i32 = pp.tile([P, E * TC], i32, tag="gli32")
            wgt = pp.tile([P, E * TC], f32, tag="wgt")
            ptr01 = pp.tile([P, 2 * NB], i32, tag="ptr01")
            wts01 = pp.tile([P, 2 * NB], f32, tag="wts01")
            oh1 = pp.tile([P, NB * E], f16, tag="oh1")
            oh2 = pp.tile([P, NB * E], f16, tag="oh2")
            mask_f16 = pp.tile([P, NB * E], f16, tag="maskf16")
            pos_all = pp.tile([P, NB * E], f16, tag="posall")
            ps32 = pp.tile([E, NB * P], f32, tag="ps32")
            combT = pp.tile([E, T], f16, tag="combT")

            bc_tok = nc.gpsimd.to_reg(T)
            bc_slot = nc.gpsimd.to_reg(E * C - 1)

            # ================= PHASE R: router =================
            w1_sbs, w2_sbs = [], []
            with (
                tc.tile_pool(name="rsb", bufs=1) as rsb,
                tc.tile_pool(name="rstr", bufs=3) as rstr,
                tc.tile_pool(name="rtmp", bufs=2) as rtmp,
            ):
                lt_sb = rsb.tile([E, T], f32, tag="ltsb")
                lg_all = rsb.tile([P, NB * E], f32, tag="lgall")
                mx_all = rsb.tile([P, NB * E], f32, tag="mxall")
                ixu_all = rsb.tile([P, NB * E], u32, tag="ixuall")

                with tc.tile_pool(name="rps", bufs=1, space="PSUM") as rps:
                    psum_lt = rps.tile([E, T], f32, tag="psumlt")
                    # strips go FIRST on the SP queue so the router isn't
                    # starved behind the 33MB weight stream
                    for kd in range(KD):
                        strip = rstr.tile([P, T], f32, tag="strip")
                        nc.sync.dma_start(
                            out=strip[:], in_=xT[kd * P : (kd + 1) * P, :]
                        )
                        for j in range(T // 512):
                            nc.tensor.matmul(
                                out=psum_lt[:, j * 512 : (j + 1) * 512],
                                lhsT=wr_sb[:, kd * E : (kd + 1) * E],
                                rhs=strip[:, j * 512 : (j + 1) * 512],
                                start=(kd == 0),
                                stop=(kd == KD - 1),
                            )
                    nc.vector.tensor_copy(out=lt_sb[:], in_=psum_lt[:])

                # expert weight streaming on the SP HWDGE queue, behind strips
                for e in range(E):
                    w1_sb = w1p.tile([P, KD * D], bf16, tag="w1sb")
                    nc.sync.dma_start(
                        out=w1_sb[:].rearrange("p (kd f) -> p kd f", kd=KD),
                        in_=w1t[e].rearrange("(kd p) f -> p kd f", p=P),
                    )
                    w2_sb = w2p.tile([P, KD * D], bf16, tag="w2sb")
                    nc.sync.dma_start(
                        out=w2_sb[:].rearrange("p (fk d) -> p fk d", fk=KD),
                        in_=w2t[e].rearrange("(fk p) d -> p fk d", p=P),
                    )
                    w1_sbs.append(w1_sb)
                    w2_sbs.append(w2_sb)

                with tc.tile_pool(name="rtr", bufs=1, space="PSUM") as rtr:
                    ptall = rtr.tile([P, NB * E], f32, tag="ptall")
                    for tb in range(NB):
                        nc.tensor.transpose(
                            out=ptall[:, tb * E : (tb + 1) * E],
                            in_=lt_sb[:, tb * P : (tb + 1) * P],
                            identity=ident_f32[:E, :E],
                        )
                    nc.vector.tensor_copy(out=lg_all[:], in_=ptall[:])

                for tb in range(NB):
                    nc.vector.max(
                        out=mx_all[:, tb * E : (tb + 1) * E],
                        in_=lg_all[:, tb * E : (tb + 1) * E],
                    )
                    nc.vector.max_index(
                        out=ixu_all[:, tb * E : (tb + 1) * E],
                        in_max=mx_all[:, tb * E : (tb + 1) * E],
                        in_values=lg_all[:, tb * E : (tb + 1) * E],
                    )

                # gate weights: wt1 = sigmoid(l1 - l2), wt2 = 1 - wt1
                d12 = rtmp.tile([P, NB], f32, tag="d12")
                nc.vector.tensor_tensor(
                    out=d12[:],
                    in0=mx_all[:].rearrange("p (t e) -> p t e", e=E)[:, :, 0],
                    in1=mx_all[:].rearrange("p (t e) -> p t e", e=E)[:, :, 1],
                    op=Alu.subtract,
                )
                nc.scalar.activation(wts01[:, 0:NB], d12[:], Act.Sigmoid)
                nc.scalar.activation(wts01[:, NB : 2 * NB], d12[:], Act.Sigmoid,
                                     scale=-1.0)
                # one-hots in f16
                ix1 = rtmp.tile([P, NB], f16, tag="ix1")
                ix2 = rtmp.tile([P, NB], f16, tag="ix2")
                nc.vector.tensor_copy(
                    out=ix1[:],
                    in_=ixu_all[:].rearrange("p (t e) -> p t e", e=E)[:, :, 0],
                )
                nc.vector.tensor_copy(
                    out=ix2[:],
                    in_=ixu_all[:].rearrange("p (t e) -> p t e", e=E)[:, :, 1],
                )
                nc.vector.tensor_tensor(
                    out=oh1[:],
                    in0=ix1[:, :, None].to_broadcast([P, NB, E]),
                    in1=iota_e[:].rearrange("p (t e) -> p t e", e=E),
                    op=Alu.is_equal,
                )
                nc.vector.tensor_tensor(
                    out=oh2[:],
                    in0=ix2[:, :, None].to_broadcast([P, NB, E]),
                    in1=iota_e[:].rearrange("p (t e) -> p t e", e=E),
                    op=Alu.is_equal,
                )
                with nc.allow_low_precision(reason="one-hot sum exact in f16"):
                    nc.vector.tensor_add(mask_f16[:], oh1[:], oh2[:])

                # ============ PHASE P: prefix-sum slot positions ============
                with (
                    tc.tile_pool(name="pps", bufs=1, space="PSUM") as pps,
                    tc.tile_pool(name="ptr2", bufs=1, space="PSUM") as ptr2,
                    tc.tile_pool(name="psb", bufs=3) as psb,
                ):
                    ps_all = pps.tile([E, NB * P], f32, tag="psall")
                    for tb in range(NB):
                        nc.tensor.matmul(
                            out=ps_all[:, tb * P : (tb + 1) * P],
                            lhsT=mask_f16[:, tb * E : (tb + 1) * E],
                            rhs=u128[:],
                            start=True,
                            stop=True,
                        )
                    nc.vector.tensor_copy(out=ps32[:], in_=ps_all[:])
                    # cross-block exclusive offsets via 16x16 strict-triangular mm
                    tot16 = psb.tile([E, NB], f16, tag="tot16")
                    nc.vector.tensor_copy(out=tot16[:], in_=ps32[:, P - 1 :: P])
                    ptot = ptr2.tile([NB, E], f16, tag="ptot")
                    nc.tensor.transpose(
                        out=ptot[:], in_=tot16[:], identity=ident_f16[:E, :E]
                    )
                    totT = psb.tile([NB, E], f16, tag="totT")
                    nc.vector.tensor_copy(out=totT[:], in_=ptot[:])
                    poff = pps.tile([E, NB], f32, tag="poff")
                    nc.tensor.matmul(
                        out=poff[:], lhsT=totT[:], rhs=u16s[:], start=True, stop=True
                    )
                    off_sb = psb.tile([E, NB], f32, tag="offsb")
                    nc.vector.tensor_copy(out=off_sb[:], in_=poff[:])

                    # add block offsets (one batched op), transpose to token-major
                    psg_all = psb.tile([E, NB * P], f16, tag="psgall")
                    nc.vector.tensor_tensor(
                        out=psg_all[:].rearrange("e (t p) -> e t p", p=P),
                        in0=ps32[:].rearrange("e (t p) -> e t p", p=P),
                        in1=off_sb[:, :, None].to_broadcast([E, NB, P]),
                        op=Alu.add,
                    )
                    pst_all = ptr2.tile([P, NB * E], f16, tag="pstall")
                    for tb in range(NB):
                        nc.tensor.transpose(
                            out=pst_all[:, tb * E : (tb + 1) * E],
                            in_=psg_all[:, tb * P : (tb + 1) * P],
                            identity=ident_f16[:E, :E],
                        )
                    nc.vector.tensor_copy(out=pos_all[:], in_=pst_all[:])

                    # slot = (pos - 1) + C*e  per choice (f16 math exact <=2048)
                    pt = psb.tile([P, NB * E], f16, tag="pt")
                    psel = psb.tile([P, NB], f16, tag="psel")
                    ek = psb.tile([P, NB], f32, tag="ek")
                    ekc = psb.tile([P, NB], f32, tag="ekc")
                    slot = psb.tile([P, NB], f32, tag="slot")
                    for k, ohk in ((0, oh1), (1, oh2)):
                        with nc.allow_low_precision(
                            reason="one-hot select; integers <=2048 exact in f16"
                        ):
                            nc.vector.tensor_mul(pt[:], pos_all[:], ohk[:])
                            nc.vector.tensor_reduce(
                                out=psel[:],
                                in_=pt[:].rearrange("p (t e) -> p t e", e=E),
                                axis=mybir.AxisListType.X, op=Alu.add,
                            )
                        nc.vector.tensor_copy(
                            out=ek[:],
                            in_=ixu_all[:].rearrange("p (t e) -> p t e", e=E)[:, :, k],
                        )
                        nc.vector.tensor_scalar(
                            out=ekc[:], in0=ek[:], scalar1=float(C),
                            scalar2=-1.0, op0=Alu.mult, op1=Alu.add,
                        )
                        nc.vector.tensor_tensor(
                            out=slot[:], in0=psel[:], in1=ekc[:], op=Alu.add
                        )
                        nc.vector.tensor_copy(
                            out=ptr01[:, k * NB : (k + 1) * NB], in_=slot[:]
                        )

                    # gate weights into the pairs tile (odd columns)
                    nc.vector.tensor_copy(
                        out=pairs[:].rearrange("p (c two) -> p c two", two=2)[:, :, 1],
                        in_=wts01[:],
                    )
                    # scatter (id, weight) pairs, one [P,1]-offset DMA per
                    # (choice, block) - HW only honors 1-column offset APs
                    scats = []
                    for col in range(2 * NB):
                        s = nc.gpsimd.indirect_dma_start(
                            out=gall2[:, :],
                            out_offset=IndirectOffsetOnAxis(
                                ap=ptr01[:, col : col + 1], axis=0
                            ),
                            in_=pairs[:, 2 * col : 2 * col + 2],
                            in_offset=None,
                            bounds_check=bc_slot,
                            oob_is_err=False,
                        )
                        tile.add_dep_helper(
                            s.ins, gall_fill.ins, sync=True, reason="fill->scat"
                        )
                        # the scatters hit disjoint slots: drop the tracker's
                        # conservative WAW chain (it would serialize them on
                        # each other's DMA-completion semaphores)
                        tc.dep_state.clear_tensor_accesses("gall2")
                        scats.append(s)
                    # barrier: ALL scatters complete (completion order across
                    # descriptor batches is not guaranteed by queue order).
                    # NOP lives on the idle SP engine - a waiting NOP on the
                    # gpsimd stream would block the Pool sequencer itself.
                    scat_gate = nc.sync.nop()
                    for s in scats:
                        tile.add_dep_helper(
                            scat_gate.ins, s.ins, sync=True, reason="scat done"
                        )
                    # readback + deinterleave
                    glw = psb.tile([P, E * TC * 2], f32, tag="glw")
                    r_gl = nc.gpsimd.dma_start(
                        out=glw[:].rearrange("p (c two) -> p c two", two=2),
                        in_=gall2.rearrange("(c p) t -> p c t", p=P),
                    )
                    tile.add_dep_helper(
                        r_gl.ins, scat_gate.ins, sync=True, reason="scat->rd"
                    )
                    nc.vector.tensor_copy(
                        out=gl_i32[:],
                        in_=glw[:].rearrange("p (c two) -> p c two", two=2)[:, :, 0],
                    )
                    nc.vector.tensor_copy(
                        out=wgt[:],
                        in_=glw[:].rearrange("p (c two) -> p c two", two=2)[:, :, 1],
                    )

                # ============ PHASE B: out base = comb @ b2 ============
                binit_writes = []
                with (
                    tc.tile_pool(name="bsbp", bufs=3) as bsbp,
                    tc.tile_pool(name="bct", bufs=1, space="PSUM") as bct,
                    tc.tile_pool(name="bps", bufs=2, space="PSUM") as bpsp,
                ):
                    comb = rsb.tile([P, NB * E], f16, tag="comb")
                    tmp = rsb.tile([P, NB * E], f16, tag="ctmp")
                    with nc.allow_low_precision(
                        reason="gate-weight one-hot mix; f16 ample for b2 term"
                    ):
                        nc.vector.tensor_tensor(
                            out=comb[:],
                            in0=oh1[:].rearrange("p (t e) -> p t e", e=E),
                            in1=wts01[:, 0:NB][:, :, None].to_broadcast([P, NB, E]),
                            op=Alu.mult,
                        )
                        nc.vector.tensor_tensor(
                            out=tmp[:],
                            in0=oh2[:].rearrange("p (t e) -> p t e", e=E),
                            in1=wts01[:, NB : 2 * NB][:, :, None].to_broadcast(
                                [P, NB, E]
                            ),
                            op=Alu.mult,
                        )
                        nc.vector.tensor_add(comb[:], comb[:], tmp[:])
                    ctp = bct.tile([E, T], f16, tag="ctp")
                    for tb in range(NB):
                        nc.tensor.transpose(
                            out=ctp[:, tb * P : (tb + 1) * P],
                            in_=comb[:, tb * E : (tb + 1) * E],
                            identity=ident_f16[:],
                        )
                    nc.vector.tensor_copy(out=combT[:], in_=ctp[:])
                    for tb in range(NB):
                        bps = bpsp.tile([P, D], f32, tag="bps")
                        for dc in range(2):
                            nc.tensor.matmul(
                                out=bps[:, dc * 512 : (dc + 1) * 512],
                                lhsT=combT[:, tb * P : (tb + 1) * P],
                                rhs=b2_sb[:, dc * 512 : (dc + 1) * 512],
                                start=True,
                                stop=True,
                            )
                        bsb = bsbp.tile([P, D], bf16, tag="bsb")
                        nc.vector.tensor_copy(out=bsb[:], in_=bps[:])
                        binit_writes.append(
                            nc.scalar.dma_start(
                                out=out[tb * P : (tb + 1) * P, :], in_=bsb[:]
                            )
                        )
                        # disjoint row blocks - no WAW chaining needed
                        tc.dep_state.clear_tensor_accesses("out")

            # ================= PHASE E: experts =================
            with (
                tc.tile_pool(name="exg", bufs=2) as exg,
                tc.tile_pool(name="ext", bufs=2) as ext,
                tc.tile_pool(name="eh", bufs=1) as eh,
                tc.tile_pool(name="ey", bufs=2) as ey,
                tc.tile_pool(name="eph", bufs=2, space="PSUM") as eph,
                tc.tile_pool(name="epy", bufs=2, space="PSUM") as epy,
            ):
                xgT_tiles = []

                def issue_gather(e):
                    # gather + transpose emitted together, two experts ahead:
                    # the xbar transposes (Act queue) run during expert e-2's
                    # W2 matmuls, so W1(e) never waits on them
                    xg = exg.tile([P, TC * D], bf16, tag="xg")
                    xgT = ext.tile([P, KD * C], bf16, tag="xgT")
                    for tcc in range(TC):
                        nc.gpsimd.indirect_dma_start(
                            out=xg[:, tcc * D : (tcc + 1) * D],
                            out_offset=None,
                            in_=xb[:, :],
                            in_offset=IndirectOffsetOnAxis(
                                ap=gl_i32[:, e * TC + tcc : e * TC + tcc + 1], axis=0
                            ),
                            bounds_check=bc_tok,
                            oob_is_err=False,
                        )
                        nc.scalar.dma_start(
                            out=xgT[:].rearrange("p (kd c) -> p kd c", kd=KD)[
                                :, :, tcc * P : (tcc + 1) * P
                            ],
                            in_=xg[:, tcc * D : (tcc + 1) * D],
                            transpose=True,
                        )
                    xgT_tiles.append(xgT)

                issue_gather(0)
                issue_gather(1)
                prev_sa = None
                for e in range(E):
                    xgT = xgT_tiles[e]
                    w1_sb = w1_sbs[e]
                    w2_sb = w2_sbs[e]
                    # h.T = gelu(W1[e].T-chunks @ x-chunks + b1)
                    hT = eh.tile([P, KD * C], bf16, tag="hT")
                    for fc in range(KD):
                        ph = eph.tile([P, C], f32, tag="ph")
                        for kd in range(KD):
                            for n0, nl in ((0, 512), (512, C - 512)):
                                nc.tensor.matmul(
                                    out=ph[:, n0 : n0 + nl],
                                    lhsT=w1_sb[:, kd * D + fc * P : kd * D + (fc + 1) * P],
                                    rhs=xgT[:, kd * C + n0 : kd * C + n0 + nl],
                                    start=(kd == 0),
                                    stop=(kd == KD - 1),
                                )
                        nc.scalar.activation(
                            hT[:, fc * C : (fc + 1) * C],
                            ph[:],
                            Act.Gelu,
                            bias=b1_sb[:, e * KD + fc : e * KD + fc + 1],
                        )
                    # y = (h @ W2[e].T) * gate_weight   (b2 via out base);
                    # each chunk's scatter-add is emitted as soon as its y
                    # rows are drained. A token's two experts share its row,
                    # so sas serialize ACROSS experts (the 5 chunks within an
                    # expert hit disjoint rows).
                    y_sb = ey.tile([P, TC * D], bf16, tag="ysb")
                    sas = []
                    for tcc in range(TC):
                        for dc in range(2):
                            py = epy.tile([P, 512], f32, tag="py")
                            for fk in range(KD):
                                nc.tensor.matmul(
                                    out=py[:],
                                    lhsT=hT[:, fk * C + tcc * P : fk * C + (tcc + 1) * P],
                                    rhs=w2_sb[:, fk * D + dc * 512 : fk * D + (dc + 1) * 512],
                                    start=(fk == 0),
                                    stop=(fk == KD - 1),
                                )
                            nc.vector.tensor_scalar(
                                out=y_sb[:, tcc * D + dc * 512 : tcc * D + (dc + 1) * 512],
                                in0=py[:],
                                scalar1=wgt[:, e * TC + tcc : e * TC + tcc + 1],
                                scalar2=None,
                                op0=Alu.mult,
                            )
                        sa = nc.gpsimd.indirect_dma_start(
                            out=out[:, :],
                            out_offset=IndirectOffsetOnAxis(
                                ap=gl_i32[:, e * TC + tcc : e * TC + tcc + 1], axis=0
                            ),
                            in_=y_sb[:, tcc * D : (tcc + 1) * D],
                            in_offset=None,
                            bounds_check=bc_tok,
                            oob_is_err=False,
                            compute_op=Alu.add,
                        )
                        if prev_sa is None:
                            for wdma in binit_writes:
                                tile.add_dep_helper(
                                    sa.ins, wdma.ins, sync=True, reason="base->sa"
                                )
                        else:
                            tile.add_dep_helper(
                                sa.ins, prev_sa.ins, sync=True, reason="sa chain"
                            )
                        tc.dep_state.clear_tensor_accesses("out")
                        sas.append(sa)
                    if e + 2 < E:
                        issue_gather(e + 2)
                    gate = nc.sync.nop()
                    for sa in sas:
                        tile.add_dep_helper(gate.ins, sa.ins, sync=True, reason="sa grp")
                    prev_sa = gate

    _split_multi_waits(nc)
    return nc


_nc_cache = None


def kernel(x, Wr, W1, b1, W2, b2):
    global _nc_cache
    if _nc_cache is None:
        _nc_cache = _build()
    nc = _nc_cache

    x = np.asarray(x, dtype=np.float32)
    Wr = np.asarray(Wr, dtype=np.float32)
    W1 = np.asarray(W1, dtype=np.float32)
    b1 = np.asarray(b1, dtype=np.float32)
    W2 = np.asarray(W2, dtype=np.float32)
    b2 = np.asarray(b2, dtype=np.float32)

    xf = x.reshape(-1, D)
    wrr_h = np.ascontiguousarray(
        Wr.T.reshape(KD, P, E).transpose(1, 0, 2).reshape(P, KD * E)
    )
    w1t_h = np.ascontiguousarray(np.transpose(W1, (0, 2, 1))).astype(ml_dtypes.bfloat16)
    w2t_h = np.ascontiguousarray(np.transpose(W2, (0, 2, 1))).astype(ml_dtypes.bfloat16)
    b1d_h = np.ascontiguousarray(
        b1.reshape(E, KD, P).transpose(2, 0, 1).reshape(P, E * KD)
    )
    b2e_h = b2.astype(np.float16)

    in_maps = []
    for i in range(N_CORES):
        s = slice(i * T, (i + 1) * T)
        xb_h = np.zeros((T + 1, D), dtype=ml_dtypes.bfloat16)
        xb_h[:T] = xf[s].astype(ml_dtypes.bfloat16)
        in_maps.append(
            {
                "xT": np.ascontiguousarray(xf[s].T),
                "xb": xb_h,
                "wrr": wrr_h,
                "w1t": w1t_h,
                "w2t": w2t_h,
                "b1d": b1d_h,
                "b2e": b2e_h,
            }
        )

    res = run_bass_kernel_spmd(nc, in_maps, core_ids=list(range(N_CORES)))
    out = np.concatenate(
        [res.results[i]["out"][:T].astype(np.float32) for i in range(N_CORES)],
        axis=0,
    ).reshape(B, L, D)
    return out
